# revision 20
# baseline (speedup 1.0000x reference)
"""DLRM forward (embedding_lookup) Trainium2 Bass kernel — v7.

Data-parallel over batch (4096/8 = 512 samples per core). Per core:
  - idx DMAs (host-reordered to (bag, table)) + indirect gathers first,
    then xt + bottom-MLP weights (bottom MLP overlaps the gather head),
    then top-MLP weights.
  - pooling folded into the PE transposes: per 128-col chunk, 4
    accumulating transpose-matmuls (one per bag slot) sum the bag in f32
    PSUM — no DVE pooling at all. Chunk j covers table slots (2j, 2j+1);
    drains write ptf entity slices with stride-4 slot slices.
  - grams per 128-sample tile (2 PSUM banks each), 4 tile_position col
    strips x alternating row bands (duplicated ptf bands).
  - merged tril extraction: one copy per (tile, res=(I-1)%4, strip) with
    free dims (chunk j, b, w) into a single zt tile [128, 7*512] with
    col = 512*j + 256*tp + 64*strip + 32*tl + 16*b + w. Cross-strip PSUM
    bank hazards (PE-W + DVE-R same bank, invisible to dataflow tracking)
    get explicit deps.
  - PSUM: one shared pool (2 bufs x 2 banks) serves MLP psums + grams
    (temporally disjoint); transpose accumulators take the other 4 banks.
  - top MLP L0 in two 256-sample halves (each gated only by its two
    tiles' extractions); L1/L2 512-wide; Sigmoid; store [1, 512]
    (permuted; host un-permutes).
No collectives needed.
"""

import numpy as np
import ml_dtypes

B, T, L, NR, M = 4096, 26, 4, 100000, 64
E27 = T + 1                      # 27 entities: slots 0..25 = tables, 26 = x
NCORES = 8
BC = B // NCORES                 # 512 samples per core
TILE = 128
NT = BC // TILE                  # 4 gather tiles per core

_BF = ml_dtypes.bfloat16

_prog_cache = {}

ZPAD = 896
NZCH = ZPAD // 128  # 7


def _ztcol(c):
    """natural in-core sample index -> permuted zt column."""
    t, u = c // 128, c % 128
    tp, tl = t // 2, t % 2
    strip, r = u // 32, u % 32
    b, w = r // 16, r % 16
    return 256 * tp + 64 * strip + 32 * tl + 16 * b + w


def build_program():
    import concourse.bass as bass
    import concourse.mybir as mybir
    import concourse.tile as tile
    from concourse import bacc
    from concourse.masks import make_identity
    from concourse.tile_rust import add_dep_helper
    from contextlib import ExitStack

    bf16 = mybir.dt.bfloat16
    f32 = mybir.dt.float32
    i32 = mybir.dt.int32
    Relu = mybir.ActivationFunctionType.Relu
    Sigmoid = mybir.ActivationFunctionType.Sigmoid

    nc = bacc.Bacc(
        "TRN2", target_bir_lowering=False, debug=False,
        num_devices=NCORES,
    )

    def din(name, shape, dt):
        return nc.dram_tensor(name, shape, dt, kind="ExternalInput").ap()

    # table as f32 container (bf16 pairs): the vector-indirect DMA path
    # quantizes index values through the transfer dtype — bf16 corrupts any
    # index > 256, f32 is exact below 2^24.
    table = din("table", [T * NR, M // 2], f32)
    xT = din("xT", [13, BC], bf16)
    idx = din("idx", [BC, L * T], i32)     # host order: (bag, table)
    wb0 = din("wb0", [13, 512], bf16)       # bot W0^T
    wb1 = din("wb1", [128, 1024], bf16)     # bot W1^T k-chunk packed
    wb2 = din("wb2", [128, 128], bf16)      # bot W2^T k-chunk packed
    wt0x = din("wt0x", [64, 512], bf16)     # top W0[:, :64]^T
    wt0z = din("wt0z", [128, NZCH * 512], bf16)  # top W0[:, 64:]^T boxed
    wt1 = din("wt1", [128, 1024], bf16)     # top W1^T k-chunk packed
    wt2 = din("wt2", [128, 2], bf16)        # top W2^T k-chunk packed
    bb0 = din("bb0", [128, 4], f32)
    bb1 = din("bb1", [128, 2], f32)
    bb2 = din("bb2", [64, 1], f32)
    bt0 = din("bt0", [128, 4], f32)
    bt1 = din("bt1", [128, 2], f32)
    bt2 = din("bt2", [1, 1], f32)
    out = nc.dram_tensor("out", [1, BC], f32, kind="ExternalOutput").ap()

    with tile.TileContext(nc) as tc, ExitStack() as ctx:
        wpool = ctx.enter_context(tc.tile_pool(name="weights", bufs=1))
        ipool = ctx.enter_context(tc.tile_pool(name="idx", bufs=2))
        ppool = ctx.enter_context(tc.tile_pool(name="ptflat", bufs=1))
        hpool = ctx.enter_context(tc.tile_pool(name="acts", bufs=1))
        zpool = ctx.enter_context(tc.tile_pool(name="ztril", bufs=1))
        opool = ctx.enter_context(tc.tile_pool(name="outs", bufs=1))
        # shared psum ring: MLP psums + gram tiles (temporally disjoint)
        pspool = ctx.enter_context(tc.tile_pool(name="ps_psum", bufs=3, space="PSUM"))
        tppool = ctx.enter_context(tc.tile_pool(name="tp_psum", bufs=1, space="PSUM"))

        # --- bottom-MLP inputs FIRST (small, ~30 KB): they must land before
        # the 6.8 MB gather monopolizes the DMA engines, so the bottom MLP
        # can run on the otherwise-idle PE during the gather head ---
        xt = wpool.tile([13, BC], bf16)
        t_wb0 = wpool.tile([13, 512], bf16)
        t_bb0 = wpool.tile([128, 4], f32)
        for t_, d_ in [(xt, xT), (t_wb0, wb0), (t_bb0, bb0)]:
            nc.sync.dma_start(t_[:], d_[:])

        # --- idx DMAs + indirect gathers ---
        # each it tile gets its own buffer: the indirect DMA's index read
        # (in_offset) must never see a reused buffer mid-gather
        its, es4s = [], []
        for t in range(NT):
            rows = slice(t * TILE, (t + 1) * TILE)
            it = wpool.tile([TILE, L * T], i32, tag=f"it{t}", name=f"it{t}")
            nc.sync.dma_start(it[:], idx[rows, :])
            its.append(it)
        # rest of the bottom MLP weights (~300 KB) before the gather issue
        t_wb1 = wpool.tile([128, 1024], bf16)
        t_wb2 = wpool.tile([128, 128], bf16)
        t_bb1 = wpool.tile([128, 2], f32)
        t_bb2 = wpool.tile([64, 1], f32)
        for t_, d_ in [(t_wb1, wb1), (t_bb1, bb1), (t_wb2, wb2), (t_bb2, bb2)]:
            nc.sync.dma_start(t_[:], d_[:])
        for t in range(NT):
            # one tag/buffer per tile: a shared tag shares one DMA semaphore,
            # which makes tile t's transposes falsely wait for tile t+2's
            # gather (sem >= 32 coarsening)
            es4 = ipool.tile([TILE, L * T * (M // 2)], f32, tag=f"es4_{t}")
            nc.gpsimd.indirect_dma_start(
                out=es4[:],
                out_offset=None,
                in_=table[:],
                in_offset=bass.IndirectOffsetOnAxis(ap=its[t][:], axis=0),
            )
            es4s.append(es4)
        t_wt0x = wpool.tile([64, 512], bf16)
        t_wt0z = wpool.tile([128, NZCH * 512], bf16)
        t_wt1 = wpool.tile([128, 1024], bf16)
        t_wt2 = wpool.tile([128, 2], bf16)
        t_bt0 = wpool.tile([128, 4], f32)
        t_bt1 = wpool.tile([128, 2], f32)
        t_bt2 = wpool.tile([1, 1], f32)
        for t_, d_ in [(t_wt0x, wt0x), (t_wt0z, wt0z), (t_wt1, wt1),
                       (t_wt2, wt2), (t_bt0, bt0), (t_bt1, bt1), (t_bt2, bt2)]:
            nc.sync.dma_start(t_[:], d_[:])
        ident = wpool.tile([128, 128], bf16)
        make_identity(nc, ident[:])

        # sample-major cols (col = c*27 + e): gram stationary APs must be
        # contiguous (strided LDWEIGHTS is fatal on HW).
        ptf = ppool.tile([64, E27 * BC], bf16, name="ptf", tag="ptf")
        ptf_r = ptf[:].rearrange("p (s e) -> p s e", e=E27)
        bxc = ppool.tile([64, BC], bf16, name="bxc", tag="bxc")
        bxp = ppool.tile([64, BC], bf16, name="bxp", tag="bxp")

        # zt per sample-half (single WRITER ENGINE per tile — same-tile
        # writes from two engines serialize cross-engine):
        # col = 256*j + 64*strip + 32*tl + 16*b + w; half 0 <- S, half 1 <- V
        zts = []
        for h in range(2):
            zth = zpool.tile([128, NZCH * 256], bf16, name=f"zt{h}", tag=f"zt{h}")
            nc.gpsimd.memset(zth[:], 0.0)
            zts.append(zth)
        zt_rs = [z[:].rearrange(
            "p (j strip tl b w) -> p j strip tl b w",
            j=NZCH, strip=4, tl=2, b=2) for z in zts]

        # --- bottom MLP, 512-wide feature-major; emitted in three stages
        # interleaved with the transpose tiles so its ACT-latency chains
        # never block the PE FIFO ---
        h0 = hpool.tile([128, 4 * BC], bf16, name="h0", tag="h0")
        h1 = hpool.tile([128, 2 * BC], bf16, name="h1", tag="h1")

        def bot_l0():
            for ob in range(4):
                ps = pspool.tile([128, BC], f32, tag="ps")
                nc.tensor.matmul(ps[:], lhsT=t_wb0[:, ob * 128:(ob + 1) * 128],
                                 rhs=xt[:], start=True, stop=True)
                nc.scalar.activation(h0[:, ob * BC:(ob + 1) * BC], ps[:],
                                     Relu, bias=t_bb0[:, ob:ob + 1])

        def bot_l1():
            for ob in range(2):
                ps = pspool.tile([128, BC], f32, tag="ps")
                for kc in range(4):
                    nc.tensor.matmul(
                        ps[:],
                        lhsT=t_wb1[:, kc * 256 + ob * 128: kc * 256 + (ob + 1) * 128],
                        rhs=h0[:, kc * BC:(kc + 1) * BC],
                        start=(kc == 0), stop=(kc == 3))
                nc.scalar.activation(h1[:, ob * BC:(ob + 1) * BC], ps[:],
                                     Relu, bias=t_bb1[:, ob:ob + 1])

        def bot_l2():
            ps = pspool.tile([64, BC], f32, tag="ps")
            for kc in range(2):
                nc.tensor.matmul(ps[:], lhsT=t_wb2[:, kc * 64:(kc + 1) * 64],
                                 rhs=h1[:, kc * BC:(kc + 1) * BC],
                                 start=(kc == 0), stop=(kc == 1))
            # x -> bxc (natural order)
            nc.scalar.activation(bxc[:], ps[:], Relu, bias=t_bb2[:, 0:1])

        def do_x_copies():
            # copies into both PTflat bands' slot 26, plus permuted bxp for
            # the top-L0 rhs (matches zt column order). Emitted after tiles
            # 0/1 so they don't head-of-line-block the drain queues.
            bxc_r = bxc[:].rearrange("p (s o) -> p s o", o=1)
            nc.vector.tensor_copy(ptf_r[:, :, 26:27], bxc_r)
            # natural col = 128*(2*tp+tl) + 32*strip + r; new col =
            # 256*tp + 64*strip + 32*tl + r  (one copy per tp: 3 free dims)
            for tp in range(2):
                src = bxc[:, tp * 256:(tp + 1) * 256].rearrange(
                    "p (tl strip r) -> p tl strip r", tl=2, strip=4)
                dst = bxp[:, tp * 256:(tp + 1) * 256].rearrange(
                    "p (strip tl r) -> p tl strip r", strip=4, tl=2)
                nc.vector.tensor_copy(dst, src)

        # --- per-tile: pooled transposes (PE accumulation) -> PTflat ---
        def do_tile(t):
            # es4 cols (bf16 view): l*1664 + slot*64 + m; chunk (l, j) is
            # 128 contiguous cols = slots (2j, 2j+1) at bag l
            es4b = es4s[t][:].bitcast(bf16)
            tp1 = tppool.tile([128, 7 * TILE], bf16, tag="tp1")
            tp2 = tppool.tile([128, 6 * TILE], bf16, tag="tp2")
            # l outer / j inner: consecutive MMs alternate tp1/tp2 banks so
            # LDWEIGHTS overlaps the accumulating transposes
            for l in range(L):
                for j in range(13):
                    k = (j // 2) if j % 2 == 0 else (7 + j // 2)
                    dst = (tp1[:, k * TILE:(k + 1) * TILE] if k < 7
                           else tp2[:, (k - 7) * TILE:(k - 6) * TILE])
                    nc.tensor.matmul(
                        dst, lhsT=es4b[:, l * 1664 + j * 128: l * 1664 + (j + 1) * 128],
                        rhs=ident[:], is_transpose=True,
                        start=(l == 0), stop=(l == L - 1))
            cols = slice(t * TILE, (t + 1) * TILE)
            # chunk at tp1 pos k came from j=2k -> tables (4k, 4k+1);
            # tp2 pos k from j=2k+1 -> tables (4k+2, 4k+3). ptf SLOT order
            # is chosen so each drain writes a contiguous slot range:
            # slot k<-table 4k, 13+k<-4k+1, 7+k<-4k+2, 20+k<-4k+3
            # (host weight perm absorbs the slot->Tcat mapping)
            s_lo1 = tp1[0:64, :].rearrange("p (k c) -> p c k", k=7)
            s_hi1 = tp1[64:128, :].rearrange("p (k c) -> p c k", k=7)
            s_lo2 = tp2[0:64, :].rearrange("p (k c) -> p c k", k=6)
            s_hi2 = tp2[64:128, :].rearrange("p (k c) -> p c k", k=6)
            pr = ptf_r
            drains = [(pr[:, cols, 0:7], s_lo1), (pr[:, cols, 13:20], s_hi1),
                      (pr[:, cols, 7:13], s_lo2), (pr[:, cols, 20:26], s_hi2)]
            # all drains on V: ptf then has a single writer engine (V),
            # so no cross-engine same-tile WAW fences at all
            for dd, ss in drains:
                nc.vector.tensor_copy(dd, ss)

        # --- gram + merged extraction per 128-sample tile ---
        # sample u = 32*strip + 16*b + w; psum col = 512*b + 32*w (+J)
        prev_ext = {}   # tile -> list of extraction instrs (for bank reuse)
        gstate = {}

        def do_gram_mms(t):
            g = pspool.tile([128, 2 * 512], f32, tag="ps")
            mm_last = {}
            for r in range(32):
                b, w = r // 16, r % 16
                col = 512 * b + 32 * w
                for strip in range(4):
                    u = 32 * strip + r
                    sap = ptf_r[:, 128 * t + u, :]
                    mm = nc.tensor.matmul(
                        g[32 * strip:32 * strip + E27, col:col + E27],
                        lhsT=sap, rhs=sap, start=True, stop=True,
                        tile_position=(0, 32 * strip))
                    mm_last[strip] = mm
                    # ps-ring rotation (bufs=3): this buffer was last read
                    # by tile t-3's extraction on V/S — PE-W vs those reads
                    # is a cross-engine bank hazard the tracker can't see
                    for e in prev_ext.get(t - 3, []):
                        add_dep_helper(mm.ins, e.ins,
                                       reason="gram MMs wait t-3 extraction (bank reuse)")
            gstate[t] = (g, mm_last)

        def do_ext(t):
            g, mm_last = gstate[t]
            tp, tl = t // 2, t % 2
            exts = []
            # merged copies: (res, strip) with free dims (j, b, w)
            # src g[32s+J, b, w, I=res+1+4j]; dst zt_h[32res+J, j, s, tl, b, w]
            # half 0 copies all on V, half 1 all on S (single writer engine)
            g_r = g[:].rearrange("p (b w i) -> p b w i", b=2, w=16)
            zt_r = zt_rs[tp]
            for res in range(4):
                nj = 7 if res < 2 else 6
                for strip in range(4):
                    src = g_r[32 * strip:32 * strip + E27, :, :,
                              res + 1::4].transpose([0, 3, 1, 2])[:, 0:nj]
                    dst = zt_r[32 * res:32 * res + E27, 0:nj, strip, tl]
                    # half 0 on S (free early, after the bMLP ACTs); half 1
                    # on V (free late) so ext3 never gates L0h1
                    if tp == 0:
                        c = nc.scalar.copy(dst, src)
                    else:
                        c = nc.vector.tensor_copy(dst, src)
                    exts.append(c)
                    # same-group cross-strip bank hazard: this copy reads
                    # both banks; MMs of other strips write them too
                    for s2 in range(4):
                        if s2 != strip:
                            add_dep_helper(c.ins, mm_last[s2].ins,
                                           reason="extraction waits all-strip grams (bank hazard)")
            prev_ext[t] = exts

        do_tile(0)
        bot_l0()
        do_tile(1)
        bot_l1()
        bot_l2()
        do_x_copies()
        do_tile(2)
        do_gram_mms(0)
        do_ext(0)
        do_gram_mms(1)
        do_ext(1)
        do_tile(3)
        do_gram_mms(2)
        do_ext(2)
        do_gram_mms(3)
        do_ext(3)

        # --- top MLP: L0 in two 256-sample halves, L1/L2 512-wide ---
        t0 = hpool.tile([128, 4 * BC], bf16, name="t0", tag="t0")
        for h in range(2):
            hc = slice(h * 256, (h + 1) * 256)
            for ob in range(4):
                ps = pspool.tile([128, 256], f32, tag="ps")
                nc.tensor.matmul(ps[:], lhsT=t_wt0x[:, ob * 128:(ob + 1) * 128],
                                 rhs=bxp[:, hc], start=True, stop=False)
                for zc in range(NZCH):
                    nc.tensor.matmul(
                        ps[:],
                        lhsT=t_wt0z[:, zc * 512 + ob * 128: zc * 512 + (ob + 1) * 128],
                        rhs=zts[h][:, zc * 256:(zc + 1) * 256],
                        start=False, stop=(zc == NZCH - 1))
                nc.scalar.activation(t0[:, ob * BC + h * 256: ob * BC + h * 256 + 256],
                                     ps[:], Relu, bias=t_bt0[:, ob:ob + 1])
        t1 = hpool.tile([128, 2 * BC], bf16, name="t1", tag="t1")
        for ob in range(2):
            ps = pspool.tile([128, BC], f32, tag="ps")
            for kc in range(4):
                nc.tensor.matmul(
                    ps[:],
                    lhsT=t_wt1[:, kc * 256 + ob * 128: kc * 256 + (ob + 1) * 128],
                    rhs=t0[:, kc * BC:(kc + 1) * BC],
                    start=(kc == 0), stop=(kc == 3))
            nc.scalar.activation(t1[:, ob * BC:(ob + 1) * BC], ps[:],
                                 Relu, bias=t_bt1[:, ob:ob + 1])
        pso = pspool.tile([1, BC], f32, tag="ps")
        for kc in range(2):
            nc.tensor.matmul(pso[:], lhsT=t_wt2[:, kc:kc + 1],
                             rhs=t1[:, kc * BC:(kc + 1) * BC],
                             start=(kc == 0), stop=(kc == 1))
        osb = opool.tile([1, BC], f32)
        nc.scalar.activation(osb[:], pso[:], Sigmoid, bias=t_bt2[:, 0:1])
        nc.sync.dma_start(out[:], osb[:])

    nc.compile()
    return nc


def _pack_k(w):
    K, N = w.shape
    return np.ascontiguousarray(
        w.reshape(K // 128, 128, N).transpose(1, 0, 2).reshape(128, -1))


def _host_inputs(dense_x, sparse_idx, emb_tables,
                 bot_W0, bot_b0, bot_W1, bot_b1, bot_W2, bot_b2,
                 top_W0, top_b0, top_W1, top_b1, top_W2, top_b2):
    f32 = np.float32
    table_bf = np.ascontiguousarray(emb_tables.reshape(T * NR, M)).astype(_BF)
    table = table_bf.view(f32)                                       # [T*NR, 32]
    flat_idx = (np.asarray(sparse_idx, dtype=np.int64)
                + (np.arange(T, dtype=np.int64) * NR)[None, :, None]).astype(np.int32)
    # kernel gather order: (bag, table)
    idx_lt = np.ascontiguousarray(
        flat_idx.reshape(B, T, L).transpose(0, 2, 1)).reshape(B, L * T)
    xTh = np.ascontiguousarray(np.asarray(dense_x, f32).T).astype(_BF)  # [13, B]

    # W0z rows into box layout under SLOT indexing (slot s = Tcat perm[s]).
    # ptf slot order: slot k<-table 4k, 13+k<-4k+1, 7+k<-4k+2, 20+k<-4k+3
    # (drain-contiguous); Tcat entity of table t is t+1, slot 26 = x = 0.
    wt0z_full = np.asarray(top_W0, f32)[:, 64:].T                     # [351, 512]
    perm = np.empty(27, dtype=np.int64)
    for k in range(7):
        perm[k] = 4 * k + 1
        perm[13 + k] = 4 * k + 2
    for k in range(6):
        perm[7 + k] = 4 * k + 3
        perm[20 + k] = 4 * k + 4
    perm[26] = 0
    wt0z_pad = np.zeros((ZPAD, 512), f32)
    for I in range(1, E27):
        for J in range(I):
            a, b = perm[I], perm[J]
            hi, lo = (a, b) if a > b else (b, a)
            p = hi * (hi - 1) // 2 + lo
            wt0z_pad[32 * (I - 1) + J] = wt0z_full[p]

    shared = {
        "table": table,
        "wb0": np.ascontiguousarray(np.asarray(bot_W0, f32).T).astype(_BF),
        "wb1": _pack_k(np.asarray(bot_W1, f32).T).astype(_BF),
        "wb2": _pack_k(np.asarray(bot_W2, f32).T).astype(_BF),
        "wt0x": np.ascontiguousarray(np.asarray(top_W0, f32)[:, :64].T).astype(_BF),
        "wt0z": _pack_k(wt0z_pad).astype(_BF),
        "wt1": _pack_k(np.asarray(top_W1, f32).T).astype(_BF),
        "wt2": _pack_k(np.asarray(top_W2, f32).T).astype(_BF),
        "bb0": np.ascontiguousarray(np.asarray(bot_b0, f32).reshape(4, 128).T),
        "bb1": np.ascontiguousarray(np.asarray(bot_b1, f32).reshape(2, 128).T),
        "bb2": np.asarray(bot_b2, f32).reshape(64, 1).copy(),
        "bt0": np.ascontiguousarray(np.asarray(top_b0, f32).reshape(4, 128).T),
        "bt1": np.ascontiguousarray(np.asarray(top_b1, f32).reshape(2, 128).T),
        "bt2": np.asarray(top_b2, f32).reshape(1, 1).copy(),
    }
    in_maps = []
    for c in range(NCORES):
        sl = slice(c * BC, (c + 1) * BC)
        m = dict(shared)
        m["xT"] = np.ascontiguousarray(xTh[:, sl])
        m["idx"] = np.ascontiguousarray(idx_lt[sl, :])
        in_maps.append(m)
    return in_maps


_ZTCOL = None


def _unpermute(core_out):
    """kernel osb columns are in zt order; map back to natural order."""
    global _ZTCOL
    if _ZTCOL is None:
        _ZTCOL = np.array([_ztcol(c) for c in range(BC)])
    return core_out[_ZTCOL]


def kernel(**inputs):
    from concourse import bass_utils

    if "prog" not in _prog_cache:
        _prog_cache["prog"] = build_program()
    nc = _prog_cache["prog"]
    in_maps = _host_inputs(**inputs)
    res = bass_utils.run_bass_kernel_spmd(nc, in_maps, core_ids=list(range(NCORES)))
    outs = [_unpermute(r["out"].reshape(BC)).reshape(BC, 1) for r in res.results]
    return np.concatenate(outs, axis=0).astype(np.float32)


if __name__ == "__main__":
    prog = build_program()
    print("program built OK")


# revision 21
# speedup vs baseline: 1.2198x; 1.2198x over previous
"""DLRM forward (embedding_lookup) Trainium2 Bass kernel — v7.

Data-parallel over batch (4096/8 = 512 samples per core). Per core:
  - idx DMAs (host-reordered to (bag, table)) + indirect gathers first,
    then xt + bottom-MLP weights (bottom MLP overlaps the gather head),
    then top-MLP weights.
  - pooling folded into the PE transposes: per 128-col chunk, 4
    accumulating transpose-matmuls (one per bag slot) sum the bag in f32
    PSUM — no DVE pooling at all. Chunk j covers table slots (2j, 2j+1);
    drains write ptf entity slices with stride-4 slot slices.
  - grams per 128-sample tile (2 PSUM banks each), 4 tile_position col
    strips x alternating row bands (duplicated ptf bands).
  - merged tril extraction: one copy per (tile, res=(I-1)%4, strip) with
    free dims (chunk j, b, w) into a single zt tile [128, 7*512] with
    col = 512*j + 256*tp + 64*strip + 32*tl + 16*b + w. Cross-strip PSUM
    bank hazards (PE-W + DVE-R same bank, invisible to dataflow tracking)
    get explicit deps.
  - PSUM: one shared pool (2 bufs x 2 banks) serves MLP psums + grams
    (temporally disjoint); transpose accumulators take the other 4 banks.
  - top MLP L0 in two 256-sample halves (each gated only by its two
    tiles' extractions); L1/L2 512-wide; Sigmoid; store [1, 512]
    (permuted; host un-permutes).
No collectives needed.
"""

import numpy as np
import ml_dtypes

B, T, L, NR, M = 4096, 26, 4, 100000, 64
E27 = T + 1                      # 27 entities: slots 0..25 = tables, 26 = x
NCORES = 8
BC = B // NCORES                 # 512 samples per core
TILE = 128
NT = BC // TILE                  # 4 gather tiles per core

_BF = ml_dtypes.bfloat16

_prog_cache = {}

ZPAD = 896
NZCH = ZPAD // 128  # 7


def _ztcol(c):
    """natural in-core sample index -> permuted zt column."""
    t, u = c // 128, c % 128
    tp, tl = t // 2, t % 2
    strip, r = u // 32, u % 32
    b, w = r // 16, r % 16
    return 256 * tp + 64 * strip + 32 * tl + 16 * b + w


def build_program():
    import concourse.bass as bass
    import concourse.mybir as mybir
    import concourse.tile as tile
    from concourse import bacc
    from concourse.masks import make_identity
    from concourse.tile_rust import add_dep_helper
    from contextlib import ExitStack

    bf16 = mybir.dt.bfloat16
    f32 = mybir.dt.float32
    i32 = mybir.dt.int32
    Relu = mybir.ActivationFunctionType.Relu
    Sigmoid = mybir.ActivationFunctionType.Sigmoid

    nc = bacc.Bacc(
        "TRN2", target_bir_lowering=False, debug=False,
        num_devices=NCORES,
    )

    def din(name, shape, dt):
        return nc.dram_tensor(name, shape, dt, kind="ExternalInput").ap()

    # table as f32 container (bf16 pairs): the vector-indirect DMA path
    # quantizes index values through the transfer dtype — bf16 corrupts any
    # index > 256, f32 is exact below 2^24.
    table = din("table", [T * NR, M // 2], f32)
    xT = din("xT", [13, BC], bf16)
    idx = din("idx", [BC, L * T], i32)     # host order: (bag, table)
    wb0 = din("wb0", [13, 512], bf16)       # bot W0^T
    wb1 = din("wb1", [128, 1024], bf16)     # bot W1^T k-chunk packed
    wb2 = din("wb2", [128, 128], bf16)      # bot W2^T k-chunk packed
    wt0x = din("wt0x", [64, 512], bf16)     # top W0[:, :64]^T
    wt0z = din("wt0z", [128, NZCH * 512], bf16)  # top W0[:, 64:]^T boxed
    wt1 = din("wt1", [128, 1024], bf16)     # top W1^T k-chunk packed
    wt2 = din("wt2", [128, 2], bf16)        # top W2^T k-chunk packed
    bb0 = din("bb0", [128, 4], f32)
    bb1 = din("bb1", [128, 2], f32)
    bb2 = din("bb2", [64, 1], f32)
    bt0 = din("bt0", [128, 4], f32)
    bt1 = din("bt1", [128, 2], f32)
    bt2 = din("bt2", [1, 1], f32)
    out = nc.dram_tensor("out", [1, BC], f32, kind="ExternalOutput").ap()

    with tile.TileContext(nc) as tc, ExitStack() as ctx:
        wpool = ctx.enter_context(tc.tile_pool(name="weights", bufs=1))
        ipool = ctx.enter_context(tc.tile_pool(name="idx", bufs=2))
        ppool = ctx.enter_context(tc.tile_pool(name="ptflat", bufs=1))
        hpool = ctx.enter_context(tc.tile_pool(name="acts", bufs=1))
        zpool = ctx.enter_context(tc.tile_pool(name="ztril", bufs=1))
        opool = ctx.enter_context(tc.tile_pool(name="outs", bufs=1))
        # shared psum ring: MLP psums + gram tiles (temporally disjoint)
        pspool = ctx.enter_context(tc.tile_pool(name="ps_psum", bufs=3, space="PSUM"))
        tppool = ctx.enter_context(tc.tile_pool(name="tp_psum", bufs=1, space="PSUM"))

        # --- bottom-MLP inputs FIRST (small, ~30 KB): they must land before
        # the 6.8 MB gather monopolizes the DMA engines, so the bottom MLP
        # can run on the otherwise-idle PE during the gather head ---
        xt = wpool.tile([13, BC], bf16)
        t_wb0 = wpool.tile([13, 512], bf16)
        t_bb0 = wpool.tile([128, 4], f32)
        for t_, d_ in [(xt, xT), (t_wb0, wb0), (t_bb0, bb0)]:
            nc.sync.dma_start(t_[:], d_[:])

        # --- idx DMAs + indirect gathers ---
        # each it tile gets its own buffer: the indirect DMA's index read
        # (in_offset) must never see a reused buffer mid-gather
        its, es4s = [], []
        for t in range(NT):
            rows = slice(t * TILE, (t + 1) * TILE)
            it = wpool.tile([TILE, L * T], i32, tag=f"it{t}", name=f"it{t}")
            nc.sync.dma_start(it[:], idx[rows, :])
            its.append(it)
        # rest of the bottom MLP weights (~300 KB) before the gather issue
        t_wb1 = wpool.tile([128, 1024], bf16)
        t_wb2 = wpool.tile([128, 128], bf16)
        t_bb1 = wpool.tile([128, 2], f32)
        t_bb2 = wpool.tile([64, 1], f32)
        for t_, d_ in [(t_wb1, wb1), (t_bb1, bb1), (t_wb2, wb2), (t_bb2, bb2)]:
            nc.sync.dma_start(t_[:], d_[:])
        prev_gather = None
        for t in range(NT):
            # one tag/buffer per tile: a shared tag shares one DMA semaphore,
            # which makes tile t's transposes falsely wait for tile t+2's
            # gather (sem >= 32 coarsening). Chain the gathers so tiles
            # complete in order (concurrent gathers all finish ~together,
            # which would stall the whole transpose pipeline on tile 0).
            es4 = ipool.tile([TILE, L * T * (M // 2)], f32, tag=f"es4_{t}")
            gi = nc.gpsimd.indirect_dma_start(
                out=es4[:],
                out_offset=None,
                in_=table[:],
                in_offset=bass.IndirectOffsetOnAxis(ap=its[t][:], axis=0),
            )
            if prev_gather is not None:
                add_dep_helper(gi.ins, prev_gather.ins,
                               reason="serialize gathers for staggered tile completion")
            prev_gather = gi
            es4s.append(es4)
        t_wt0x = wpool.tile([64, 512], bf16)
        t_wt0z = wpool.tile([128, NZCH * 512], bf16)
        t_wt1 = wpool.tile([128, 1024], bf16)
        t_wt2 = wpool.tile([128, 2], bf16)
        t_bt0 = wpool.tile([128, 4], f32)
        t_bt1 = wpool.tile([128, 2], f32)
        t_bt2 = wpool.tile([1, 1], f32)
        for t_, d_ in [(t_wt0x, wt0x), (t_wt0z, wt0z), (t_wt1, wt1),
                       (t_wt2, wt2), (t_bt0, bt0), (t_bt1, bt1), (t_bt2, bt2)]:
            nc.sync.dma_start(t_[:], d_[:])
        ident = wpool.tile([128, 128], bf16)
        make_identity(nc, ident[:])

        # sample-major cols (col = c*27 + e): gram stationary APs must be
        # contiguous (strided LDWEIGHTS is fatal on HW).
        ptf = ppool.tile([64, E27 * BC], bf16, name="ptf", tag="ptf")
        ptf_r = ptf[:].rearrange("p (s e) -> p s e", e=E27)
        bxc = ppool.tile([64, BC], bf16, name="bxc", tag="bxc")
        bxp = ppool.tile([64, BC], bf16, name="bxp", tag="bxp")

        # zt per sample-half (single WRITER ENGINE per tile — same-tile
        # writes from two engines serialize cross-engine):
        # col = 256*j + 64*strip + 32*tl + 16*b + w; half 0 <- S, half 1 <- V
        zts = []
        for h in range(2):
            zth = zpool.tile([128, NZCH * 256], bf16, name=f"zt{h}", tag=f"zt{h}")
            nc.gpsimd.memset(zth[:], 0.0)
            zts.append(zth)
        zt_rs = [z[:].rearrange(
            "p (j strip tl b w) -> p j strip tl b w",
            j=NZCH, strip=4, tl=2, b=2) for z in zts]

        # --- bottom MLP, 512-wide feature-major; emitted in three stages
        # interleaved with the transpose tiles so its ACT-latency chains
        # never block the PE FIFO ---
        h0 = hpool.tile([128, 4 * BC], bf16, name="h0", tag="h0")
        h1 = hpool.tile([128, 2 * BC], bf16, name="h1", tag="h1")

        def bot_l0():
            for ob in range(4):
                ps = pspool.tile([128, BC], f32, tag="ps")
                nc.tensor.matmul(ps[:], lhsT=t_wb0[:, ob * 128:(ob + 1) * 128],
                                 rhs=xt[:], start=True, stop=True)
                nc.scalar.activation(h0[:, ob * BC:(ob + 1) * BC], ps[:],
                                     Relu, bias=t_bb0[:, ob:ob + 1])

        def bot_l1():
            for ob in range(2):
                ps = pspool.tile([128, BC], f32, tag="ps")
                for kc in range(4):
                    nc.tensor.matmul(
                        ps[:],
                        lhsT=t_wb1[:, kc * 256 + ob * 128: kc * 256 + (ob + 1) * 128],
                        rhs=h0[:, kc * BC:(kc + 1) * BC],
                        start=(kc == 0), stop=(kc == 3))
                nc.scalar.activation(h1[:, ob * BC:(ob + 1) * BC], ps[:],
                                     Relu, bias=t_bb1[:, ob:ob + 1])

        def bot_l2():
            ps = pspool.tile([64, BC], f32, tag="ps")
            for kc in range(2):
                nc.tensor.matmul(ps[:], lhsT=t_wb2[:, kc * 64:(kc + 1) * 64],
                                 rhs=h1[:, kc * BC:(kc + 1) * BC],
                                 start=(kc == 0), stop=(kc == 1))
            # x -> bxc (natural order)
            nc.scalar.activation(bxc[:], ps[:], Relu, bias=t_bb2[:, 0:1])

        def do_x_copies():
            # copies into both PTflat bands' slot 26, plus permuted bxp for
            # the top-L0 rhs (matches zt column order). Emitted after tiles
            # 0/1 so they don't head-of-line-block the drain queues.
            bxc_r = bxc[:].rearrange("p (s o) -> p s o", o=1)
            nc.vector.tensor_copy(ptf_r[:, :, 26:27], bxc_r)
            # natural col = 128*(2*tp+tl) + 32*strip + r; new col =
            # 256*tp + 64*strip + 32*tl + r  (one copy per tp: 3 free dims)
            for tp in range(2):
                src = bxc[:, tp * 256:(tp + 1) * 256].rearrange(
                    "p (tl strip r) -> p tl strip r", tl=2, strip=4)
                dst = bxp[:, tp * 256:(tp + 1) * 256].rearrange(
                    "p (strip tl r) -> p tl strip r", strip=4, tl=2)
                nc.vector.tensor_copy(dst, src)

        # --- per-tile: pooled transposes (PE accumulation) -> PTflat ---
        def do_tile(t):
            # es4 cols (bf16 view): l*1664 + slot*64 + m; chunk (l, j) is
            # 128 contiguous cols = slots (2j, 2j+1) at bag l
            es4b = es4s[t][:].bitcast(bf16)
            tp1 = tppool.tile([128, 7 * TILE], bf16, tag="tp1")
            tp2 = tppool.tile([128, 6 * TILE], bf16, tag="tp2")
            # l outer / j inner: consecutive MMs alternate tp1/tp2 banks so
            # LDWEIGHTS overlaps the accumulating transposes
            for l in range(L):
                for j in range(13):
                    k = (j // 2) if j % 2 == 0 else (7 + j // 2)
                    dst = (tp1[:, k * TILE:(k + 1) * TILE] if k < 7
                           else tp2[:, (k - 7) * TILE:(k - 6) * TILE])
                    nc.tensor.matmul(
                        dst, lhsT=es4b[:, l * 1664 + j * 128: l * 1664 + (j + 1) * 128],
                        rhs=ident[:], is_transpose=True,
                        start=(l == 0), stop=(l == L - 1))
            cols = slice(t * TILE, (t + 1) * TILE)
            # chunk at tp1 pos k came from j=2k -> tables (4k, 4k+1);
            # tp2 pos k from j=2k+1 -> tables (4k+2, 4k+3). ptf SLOT order
            # is chosen so each drain writes a contiguous slot range:
            # slot k<-table 4k, 13+k<-4k+1, 7+k<-4k+2, 20+k<-4k+3
            # (host weight perm absorbs the slot->Tcat mapping)
            s_lo1 = tp1[0:64, :].rearrange("p (k c) -> p c k", k=7)
            s_hi1 = tp1[64:128, :].rearrange("p (k c) -> p c k", k=7)
            s_lo2 = tp2[0:64, :].rearrange("p (k c) -> p c k", k=6)
            s_hi2 = tp2[64:128, :].rearrange("p (k c) -> p c k", k=6)
            pr = ptf_r
            drains = [(pr[:, cols, 0:7], s_lo1), (pr[:, cols, 13:20], s_hi1),
                      (pr[:, cols, 7:13], s_lo2), (pr[:, cols, 20:26], s_hi2)]
            # all drains on V: ptf then has a single writer engine (V),
            # so no cross-engine same-tile WAW fences at all
            for dd, ss in drains:
                nc.vector.tensor_copy(dd, ss)

        # --- gram + merged extraction per 128-sample tile ---
        # sample u = 32*strip + 16*b + w; psum col = 512*b + 32*w (+J)
        prev_ext = {}   # tile -> list of extraction instrs (for bank reuse)
        gstate = {}

        def do_gram_mms(t):
            g = pspool.tile([128, 2 * 512], f32, tag="ps")
            mm_last = {}
            for r in range(32):
                b, w = r // 16, r % 16
                col = 512 * b + 32 * w
                for strip in range(4):
                    u = 32 * strip + r
                    sap = ptf_r[:, 128 * t + u, :]
                    mm = nc.tensor.matmul(
                        g[32 * strip:32 * strip + E27, col:col + E27],
                        lhsT=sap, rhs=sap, start=True, stop=True,
                        tile_position=(0, 32 * strip))
                    mm_last[strip] = mm
                    # ps-ring rotation (bufs=3): this buffer was last read
                    # by tile t-3's extraction on V/S — PE-W vs those reads
                    # is a cross-engine bank hazard the tracker can't see
                    for e in prev_ext.get(t - 3, []):
                        add_dep_helper(mm.ins, e.ins,
                                       reason="gram MMs wait t-3 extraction (bank reuse)")
            gstate[t] = (g, mm_last)

        def do_ext(t):
            g, mm_last = gstate[t]
            tp, tl = t // 2, t % 2
            exts = []
            # merged copies: (res, strip) with free dims (j, b, w)
            # src g[32s+J, b, w, I=res+1+4j]; dst zt_h[32res+J, j, s, tl, b, w]
            # half 0 copies all on V, half 1 all on S (single writer engine)
            g_r = g[:].rearrange("p (b w i) -> p b w i", b=2, w=16)
            zt_r = zt_rs[tp]
            for res in range(4):
                nj = 7 if res < 2 else 6
                for strip in range(4):
                    src = g_r[32 * strip:32 * strip + E27, :, :,
                              res + 1::4].transpose([0, 3, 1, 2])[:, 0:nj]
                    dst = zt_r[32 * res:32 * res + E27, 0:nj, strip, tl]
                    # half 0 on S (free early, after the bMLP ACTs); half 1
                    # on V (free late) so ext3 never gates L0h1
                    if tp == 0:
                        c = nc.scalar.copy(dst, src)
                    else:
                        c = nc.vector.tensor_copy(dst, src)
                    exts.append(c)
                    # same-group cross-strip bank hazard: this copy reads
                    # both banks; MMs of other strips write them too
                    for s2 in range(4):
                        if s2 != strip:
                            add_dep_helper(c.ins, mm_last[s2].ins,
                                           reason="extraction waits all-strip grams (bank hazard)")
            prev_ext[t] = exts

        do_tile(0)
        bot_l0()
        do_tile(1)
        bot_l1()
        bot_l2()
        do_x_copies()
        do_tile(2)
        do_gram_mms(0)
        do_ext(0)
        do_gram_mms(1)
        do_ext(1)
        do_tile(3)
        do_gram_mms(2)
        do_ext(2)
        do_gram_mms(3)
        do_ext(3)

        # --- top MLP: L0 in two 256-sample halves, L1/L2 512-wide ---
        t0 = hpool.tile([128, 4 * BC], bf16, name="t0", tag="t0")
        for h in range(2):
            hc = slice(h * 256, (h + 1) * 256)
            for ob in range(4):
                ps = pspool.tile([128, 256], f32, tag="ps")
                nc.tensor.matmul(ps[:], lhsT=t_wt0x[:, ob * 128:(ob + 1) * 128],
                                 rhs=bxp[:, hc], start=True, stop=False)
                for zc in range(NZCH):
                    nc.tensor.matmul(
                        ps[:],
                        lhsT=t_wt0z[:, zc * 512 + ob * 128: zc * 512 + (ob + 1) * 128],
                        rhs=zts[h][:, zc * 256:(zc + 1) * 256],
                        start=False, stop=(zc == NZCH - 1))
                nc.scalar.activation(t0[:, ob * BC + h * 256: ob * BC + h * 256 + 256],
                                     ps[:], Relu, bias=t_bt0[:, ob:ob + 1])
        t1 = hpool.tile([128, 2 * BC], bf16, name="t1", tag="t1")
        for ob in range(2):
            ps = pspool.tile([128, BC], f32, tag="ps")
            for kc in range(4):
                nc.tensor.matmul(
                    ps[:],
                    lhsT=t_wt1[:, kc * 256 + ob * 128: kc * 256 + (ob + 1) * 128],
                    rhs=t0[:, kc * BC:(kc + 1) * BC],
                    start=(kc == 0), stop=(kc == 3))
            nc.scalar.activation(t1[:, ob * BC:(ob + 1) * BC], ps[:],
                                 Relu, bias=t_bt1[:, ob:ob + 1])
        pso = pspool.tile([1, BC], f32, tag="ps")
        for kc in range(2):
            nc.tensor.matmul(pso[:], lhsT=t_wt2[:, kc:kc + 1],
                             rhs=t1[:, kc * BC:(kc + 1) * BC],
                             start=(kc == 0), stop=(kc == 1))
        osb = opool.tile([1, BC], f32)
        nc.scalar.activation(osb[:], pso[:], Sigmoid, bias=t_bt2[:, 0:1])
        nc.sync.dma_start(out[:], osb[:])

    nc.compile()
    return nc


def _pack_k(w):
    K, N = w.shape
    return np.ascontiguousarray(
        w.reshape(K // 128, 128, N).transpose(1, 0, 2).reshape(128, -1))


def _host_inputs(dense_x, sparse_idx, emb_tables,
                 bot_W0, bot_b0, bot_W1, bot_b1, bot_W2, bot_b2,
                 top_W0, top_b0, top_W1, top_b1, top_W2, top_b2):
    f32 = np.float32
    table_bf = np.ascontiguousarray(emb_tables.reshape(T * NR, M)).astype(_BF)
    table = table_bf.view(f32)                                       # [T*NR, 32]
    flat_idx = (np.asarray(sparse_idx, dtype=np.int64)
                + (np.arange(T, dtype=np.int64) * NR)[None, :, None]).astype(np.int32)
    # kernel gather order: (bag, table)
    idx_lt = np.ascontiguousarray(
        flat_idx.reshape(B, T, L).transpose(0, 2, 1)).reshape(B, L * T)
    xTh = np.ascontiguousarray(np.asarray(dense_x, f32).T).astype(_BF)  # [13, B]

    # W0z rows into box layout under SLOT indexing (slot s = Tcat perm[s]).
    # ptf slot order: slot k<-table 4k, 13+k<-4k+1, 7+k<-4k+2, 20+k<-4k+3
    # (drain-contiguous); Tcat entity of table t is t+1, slot 26 = x = 0.
    wt0z_full = np.asarray(top_W0, f32)[:, 64:].T                     # [351, 512]
    perm = np.empty(27, dtype=np.int64)
    for k in range(7):
        perm[k] = 4 * k + 1
        perm[13 + k] = 4 * k + 2
    for k in range(6):
        perm[7 + k] = 4 * k + 3
        perm[20 + k] = 4 * k + 4
    perm[26] = 0
    wt0z_pad = np.zeros((ZPAD, 512), f32)
    for I in range(1, E27):
        for J in range(I):
            a, b = perm[I], perm[J]
            hi, lo = (a, b) if a > b else (b, a)
            p = hi * (hi - 1) // 2 + lo
            wt0z_pad[32 * (I - 1) + J] = wt0z_full[p]

    shared = {
        "table": table,
        "wb0": np.ascontiguousarray(np.asarray(bot_W0, f32).T).astype(_BF),
        "wb1": _pack_k(np.asarray(bot_W1, f32).T).astype(_BF),
        "wb2": _pack_k(np.asarray(bot_W2, f32).T).astype(_BF),
        "wt0x": np.ascontiguousarray(np.asarray(top_W0, f32)[:, :64].T).astype(_BF),
        "wt0z": _pack_k(wt0z_pad).astype(_BF),
        "wt1": _pack_k(np.asarray(top_W1, f32).T).astype(_BF),
        "wt2": _pack_k(np.asarray(top_W2, f32).T).astype(_BF),
        "bb0": np.ascontiguousarray(np.asarray(bot_b0, f32).reshape(4, 128).T),
        "bb1": np.ascontiguousarray(np.asarray(bot_b1, f32).reshape(2, 128).T),
        "bb2": np.asarray(bot_b2, f32).reshape(64, 1).copy(),
        "bt0": np.ascontiguousarray(np.asarray(top_b0, f32).reshape(4, 128).T),
        "bt1": np.ascontiguousarray(np.asarray(top_b1, f32).reshape(2, 128).T),
        "bt2": np.asarray(top_b2, f32).reshape(1, 1).copy(),
    }
    in_maps = []
    for c in range(NCORES):
        sl = slice(c * BC, (c + 1) * BC)
        m = dict(shared)
        m["xT"] = np.ascontiguousarray(xTh[:, sl])
        m["idx"] = np.ascontiguousarray(idx_lt[sl, :])
        in_maps.append(m)
    return in_maps


_ZTCOL = None


def _unpermute(core_out):
    """kernel osb columns are in zt order; map back to natural order."""
    global _ZTCOL
    if _ZTCOL is None:
        _ZTCOL = np.array([_ztcol(c) for c in range(BC)])
    return core_out[_ZTCOL]


def kernel(**inputs):
    from concourse import bass_utils

    if "prog" not in _prog_cache:
        _prog_cache["prog"] = build_program()
    nc = _prog_cache["prog"]
    in_maps = _host_inputs(**inputs)
    res = bass_utils.run_bass_kernel_spmd(nc, in_maps, core_ids=list(range(NCORES)))
    outs = [_unpermute(r["out"].reshape(BC)).reshape(BC, 1) for r in res.results]
    return np.concatenate(outs, axis=0).astype(np.float32)


if __name__ == "__main__":
    prog = build_program()
    print("program built OK")
